# revision 42
# baseline (speedup 1.0000x reference)
"""GCN (2-layer + linear classifier) on 8 Trainium2 NeuronCores.

Math: with A = adjacency+self-loops and dis = deg^-1/2 (deg over incoming
edges incl. self-loops), PyG gcn_norm gives norm_e = dis[src]*dis[dst], which
is separable. So each conv layer is
    out = dis ⊙ (A_binary @ ((dis ⊙ h) @ W)) + b
i.e. a plain binary-adjacency segment-sum around a dense matmul — no per-edge
scaling. Layer 1 additionally commutes the dense matmul past the
aggregation:  h1 = dis ⊙ ((A @ (dis ⊙ x)) @ W1) + b1, so the layer-1 gather
reads raw x-tilde rows (a kernel *input*) and no message table has to be
built on device before aggregation can start.

Distribution (8 cores): nodes are split into 8 contiguous chunks; edges are
partitioned by destination-node owner (segment-sum is local); the single
cross-core exchange is one fp16 AllGather of the layer-2 message table
(rows = (dis ⊙ relu(h1)) @ W2), issued in slab-aligned chunks right after
the last contributing window's epilogue so the collective overlaps the
layer-1 aggregation tail. Layer-2 gathers read the collective's Shared
output buffer directly (no staging copy).

Per core, the aggregation runs per 128-destination-node window: source rows
are fetched from a DRAM fp16 row table with dma_gather (128 rows/tile),
reduced onto a PSUM accumulator with TensorEngine matmuls against one-hot
selection matrices built by a DVE is_equal, then the epilogue (dis-scale,
bias, relu, next-layer matmul) runs on DVE/ACT/PE. Node indices are split
into lo/hi tables at row 32768 because dma_gather indices are int16.
"""
import os
import numpy as np

import concourse.bacc as bacc
import concourse.bass as bass
import concourse.mybir as mybir
import concourse.tile as tile
from concourse import library_config
from concourse.bass_utils import run_bass_kernel_spmd

N_CORES = 8
D = 128           # feature dim (= hidden dim = partition count)
LO_DEFAULT = 32768

fp16 = mybir.dt.float16
f32 = mybir.dt.float32
i16 = mybir.dt.int16


# ---------------------------------------------------------------- host prep

def _wrap16(v):
    """dma_gather index layout: idx i -> partition i%16, col i//16,
    replicated across all eight 16-partition groups."""
    a = v.reshape(-1, 16).T.astype(np.int16)
    return np.tile(a, (8, 1))


def _layer_meta(w_global, wrow_all, pos, n_cores, WPC, lo_rows):
    """Per-layer gather metadata: edges live at table position pos[e]; split
    lo/hi at lo_rows, sort by (core-window, half), emit per-core idx tables
    (tile-rounded, pad idx 0) + wrow tables (pad -1), shared per-window tile
    counts T_lo/T_hi and real (max-over-cores) index counts n_lo/n_hi."""
    hi_flag = (pos >= lo_rows).astype(np.int64)
    key = w_global * 2 + hi_flag
    # within each (window, half) segment, order edges by table position so
    # the gather descriptor stream sweeps HBM addresses monotonically
    order = np.lexsort((pos, key))
    counts = np.bincount(key, minlength=n_cores * WPC * 2).reshape(n_cores, WPC, 2)
    offs = np.concatenate([[0], np.cumsum(counts.reshape(-1))]).astype(np.int64)

    n_lo = [int(counts[:, w, 0].max()) for w in range(WPC)]
    n_hi = [int(counts[:, w, 1].max()) for w in range(WPC)]
    T_lo = [-(-n // 128) for n in n_lo]
    T_hi = [-(-n // 128) for n in n_hi]

    pos_sorted = pos[order]
    wrow_sorted = wrow_all[order]

    per_core = []
    for c in range(n_cores):
        ilo_parts, ihi_parts, wr_parts = [], [], []
        for w in range(WPC):
            base = (c * WPC + w) * 2
            for h, T in ((0, T_lo[w]), (1, T_hi[w])):
                n = T * 128
                if n == 0:
                    continue
                a, b = offs[base + h], offs[base + h + 1]
                sp = pos_sorted[a:b]
                wr = wrow_sorted[a:b]
                pad = n - (b - a)
                idx = np.concatenate([sp - (lo_rows if h else 0),
                                      np.zeros(pad, np.int64)]).astype(np.int16)
                wrc = np.concatenate([wr, np.full(pad, -1.0, np.float16)])
                (ihi_parts if h else ilo_parts).append(idx)
                wr_parts.append(wrc.reshape(T, 128).T)
        idx_lo = _wrap16(np.concatenate(ilo_parts)) if ilo_parts else np.zeros((128, 8), np.int16)
        idx_hi = _wrap16(np.concatenate(ihi_parts)) if ihi_parts else np.zeros((128, 8), np.int16)
        wrow_c = np.concatenate(wr_parts, axis=1).astype(np.float16)
        per_core.append((idx_lo, idx_hi, wrow_c))

    meta = dict(T_lo=tuple(T_lo), T_hi=tuple(T_hi),
                n_lo=tuple(n_lo), n_hi=tuple(n_hi))
    return meta, per_core


def _balance_windows(v, WPC):
    """Assign locN nodes (rows of v = per-node edge-count vectors) to WPC
    windows of 128 slots, equalizing each count dimension across windows.
    Greedy: nodes in descending total count, each to the non-full window
    with the smallest resulting max load ratio. Returns permrow[node] =
    window*128 + row."""
    locN = v.shape[0]
    tgt = v.sum(axis=0) / WPC + 1e-9
    order = np.argsort(-v.sum(axis=1), kind="stable")
    cur = np.zeros((WPC, v.shape[1]))
    cnt = np.zeros(WPC, np.int64)
    permrow = np.empty(locN, np.int64)
    for i in order:
        ratio = ((cur + v[i]) / tgt).max(axis=1)
        ratio[cnt >= 128] = np.inf
        w = int(np.argmin(ratio))
        permrow[i] = w * 128 + cnt[w]
        cur[w] += v[i]
        cnt[w] += 1
    return permrow


def prep(x, edge_index, n_cores=N_CORES, lo_rows=LO_DEFAULT):
    N = x.shape[0]
    locN = N // n_cores
    assert locN * n_cores == N
    WPC = -(-locN // 128)              # real (dst) windows per core
    NLOC = -(-locN // 512) * 512       # padded nodes per core (512-slab aligned)
    NPAD = n_cores * NLOC
    nslab = NLOC // 512
    assert lo_rows % 128 == 0 and lo_rows < 32768 + 1

    src_all = np.asarray(edge_index[0]).astype(np.int64)
    dst_all = np.asarray(edge_index[1]).astype(np.int64)

    # degree includes the implicit self-loop; the loops themselves are NOT in
    # the gather lists — each window's self-loop block is read directly from
    # the local own-chunk table and applied via an identity matmul
    deg = (np.bincount(dst_all, minlength=N) + 1).astype(np.float32)

    # ---- per-core dst->window assignment, balancing each window's lo/hi
    # edge counts (both layers' geometries) so the shared max-over-cores
    # tile profile carries almost no padding. Sources keep identity layout;
    # only the within-core (window,row) labeling of nodes changes, so this
    # is free on device. permrow_all[n] = padded local position of node n.
    src_core = src_all // locN
    src_loc = src_all - src_core * locN
    # provisional lo/hi membership from the identity layout (exact for every
    # source core except the one the 32768 boundary cuts through, whose few
    # boundary nodes add only ~5 misclassified edges per window)
    hi1_e = (src_core * NLOC + src_loc >= lo_rows).astype(np.int64)

    permrow_all = np.empty(N, np.int64)
    for c in range(n_cores):
        sel = (dst_all // locN) == c
        dloc = dst_all[sel] - c * locN
        v = np.zeros((locN, 2))
        np.add.at(v[:, 0], dloc, 1.0 - hi1_e[sel])
        np.add.at(v[:, 1], dloc, hi1_e[sel].astype(np.float64))
        permrow_all[c * locN:(c + 1) * locN] = _balance_windows(v, WPC)

    d_core = dst_all // locN
    d_pos = permrow_all[dst_all]
    w_global = d_core * WPC + d_pos // 128
    wrow_all = (d_pos % 128).astype(np.float16)
    # source padded id in the permuted layout (layer-1 table position)
    spid = src_core * NLOC + permrow_all[src_all]

    meta1, pc1 = _layer_meta(w_global, wrow_all, spid, n_cores, WPC, lo_rows)

    # layer-2 table = the AllGather output laid out chunk-major then
    # core-major: chunk k covers local windows [spl[k], spl[k+1]) of every
    # core (window granularity; tiny last chunk to shorten the layer
    # transition tail)
    spl = sorted({0, WPC * 16 // 49, WPC * 28 // 49, WPC * 36 // 49,
                  WPC * 44 // 49, WPC - 1, WPC})
    ag_base = np.zeros(len(spl) - 1, np.int64)
    for k in range(1, len(spl) - 1):
        ag_base[k] = ag_base[k - 1] + n_cores * (spl[k] - spl[k - 1]) * 128
    win_of = spid % NLOC // 128
    chunk_of = np.searchsorted(np.asarray(spl[1:]), win_of, side="right")
    ck_rows = (np.asarray(spl)[chunk_of + 1] - np.asarray(spl)[chunk_of]) * 128
    pos2 = (ag_base[chunk_of] + (spid // NLOC) * ck_rows
            + spid % NLOC - np.asarray(spl)[chunk_of] * 128)

    meta2, pc2 = _layer_meta(w_global, wrow_all, pos2, n_cores, WPC, lo_rows)

    per_core = []
    dis = 1.0 / np.sqrt(deg)
    for c in range(n_cores):
        pr = permrow_all[c * locN:(c + 1) * locN]
        dr = np.ones((1, NLOC), np.float32)
        dr[0, pr] = dis[c * locN:(c + 1) * locN]
        per_core.append(dict(
            idx_lo1=pc1[c][0], idx_hi1=pc1[c][1], wrow1=pc1[c][2],
            idx_lo2=pc2[c][0], idx_hi2=pc2[c][1], wrow2=pc2[c][2],
            dis_row=dr, permrow=pr))

    # x-tilde ROW table, fp16: (dis * x) laid out [NPAD, D] in the permuted
    # (window-balanced) per-core order
    xrows = np.zeros((NPAD, D), np.float16)
    xs = (np.asarray(x, np.float32) * dis[:, None]).astype(np.float16)
    for c in range(n_cores):
        pr = per_core[c]["permrow"]
        xrows[c * NLOC + pr] = xs[c * locN:(c + 1) * locN]
        per_core[c]["xown"] = np.ascontiguousarray(xrows[c * NLOC:(c + 1) * NLOC])

    struct = dict(N=N, locN=locN, WPC=WPC, NLOC=NLOC, NPAD=NPAD,
                  lo_rows=lo_rows, n_cores=n_cores, spl=tuple(spl),
                  m1=tuple(sorted(meta1.items())), m2=tuple(sorted(meta2.items())))
    return struct, per_core, xrows


# ------------------------------------------------------------- bass program

def build(struct):
    WPC, NLOC, NPAD = struct["WPC"], struct["NLOC"], struct["NPAD"]
    LO = struct["lo_rows"]
    n_cores = struct["n_cores"]
    spl = list(struct["spl"])
    m1 = dict(struct["m1"])
    m2 = dict(struct["m2"])

    def csz(m):
        return (max(8, 8 * sum(m["T_lo"])), max(8, 8 * sum(m["T_hi"])),
                sum(m["T_lo"]) + sum(m["T_hi"]))
    CL1, CH1, TT1 = csz(m1)
    CL2, CH2, TT2 = csz(m2)
    maxT = max(max(m["T_lo"][w] + m["T_hi"][w] for w in range(WPC))
               for m in (m1, m2))

    nc = bacc.Bacc("TRN2", target_bir_lowering=False, debug=False,
                   num_devices=n_cores, num_swdge_queues=4,
                   dynamic_dma_scratch_size=49152)
    xtab_lo_d = nc.dram_tensor("xtab_lo", [LO, D], fp16, kind="ExternalInput")
    xtab_hi_d = nc.dram_tensor("xtab_hi", [NPAD - LO, D], fp16, kind="ExternalInput")
    xown_d = nc.dram_tensor("xown", [NLOC, D], fp16, kind="ExternalInput")
    W1_d = nc.dram_tensor("W1h", [D, D], fp16, kind="ExternalInput")
    W2_d = nc.dram_tensor("W2h", [D, D], fp16, kind="ExternalInput")
    Wc_d = nc.dram_tensor("Wch", [D, 2], fp16, kind="ExternalInput")
    b1_d = nc.dram_tensor("b1c", [D, 1], f32, kind="ExternalInput")
    b2_d = nc.dram_tensor("b2c", [D, 1], f32, kind="ExternalInput")
    bc_d = nc.dram_tensor("bcrep", [D, 2], f32, kind="ExternalInput")
    iota_d = nc.dram_tensor("iota", [D, 8 * D], fp16, kind="ExternalInput")
    ident_d = nc.dram_tensor("ident", [D, D], fp16, kind="ExternalInput")
    dis_d = nc.dram_tensor("dis_row", [1, NLOC], f32, kind="ExternalInput")
    ilo1_d = nc.dram_tensor("idx_lo1", [128, CL1], i16, kind="ExternalInput")
    ihi1_d = nc.dram_tensor("idx_hi1", [128, CH1], i16, kind="ExternalInput")
    wrow1_d = nc.dram_tensor("wrow1", [128, TT1], fp16, kind="ExternalInput")
    ilo2_d = nc.dram_tensor("idx_lo2", [128, CL2], i16, kind="ExternalInput")
    ihi2_d = nc.dram_tensor("idx_hi2", [128, CH2], i16, kind="ExternalInput")
    wrow2_d = nc.dram_tensor("wrow2", [128, TT2], fp16, kind="ExternalInput")
    out_d = nc.dram_tensor("out", [NLOC, 2], f32, kind="ExternalOutput")

    ag_in = nc.dram_tensor("ag_in", [NLOC, D], fp16)
    # one Shared buffer holding the whole all-gathered layer-2 table,
    # chunk-major then core-major; layer-2 gathers read it directly
    AGR = n_cores * WPC * 128
    ag_all = nc.dram_tensor("ag_all", [AGR, D], fp16, addr_space="Shared")

    with tile.TileContext(nc) as tc:
        nc.gpsimd.load_library(library_config.mlp)
        with (
            tc.tile_pool(name="const", bufs=1) as cp,
            tc.tile_pool(name="work", bufs=4) as wp,
            tc.tile_pool(name="msgp", bufs=2) as mp,
            tc.tile_pool(name="Sp", bufs=4) as sp_,
            tc.tile_pool(name="psum", bufs=2, space="PSUM") as pp,
        ):
            # ---- gather metadata first: layer-1 gathers depend only on these
            ilo1 = cp.tile([128, CL1], i16)
            ihi1 = cp.tile([128, CH1], i16)
            wro1 = cp.tile([128, TT1], fp16)
            nc.sync.dma_start(out=ilo1[:], in_=ilo1_d[:])
            nc.sync.dma_start(out=ihi1[:], in_=ihi1_d[:])
            nc.sync.dma_start(out=wro1[:], in_=wrow1_d[:])

            # ---- constants
            W1s = cp.tile([D, D], fp16)
            W2s = cp.tile([D, D], fp16)
            Wcs = cp.tile([D, 2], fp16)
            nc.sync.dma_start(out=W1s[:], in_=W1_d[:])
            nc.sync.dma_start(out=W2s[:], in_=W2_d[:])
            nc.sync.dma_start(out=Wcs[:], in_=Wc_d[:])
            ident = cp.tile([D, D], fp16)
            nc.sync.dma_start(out=ident[:], in_=ident_d[:])
            b1c = cp.tile([D, 1], f32)
            b2c = cp.tile([D, 1], f32)
            bcr = cp.tile([D, 2], f32)
            iota = cp.tile([D, 8 * D], fp16)
            nc.sync.dma_start(out=b1c[:], in_=b1_d[:])
            nc.sync.dma_start(out=b2c[:], in_=b2_d[:])
            nc.sync.dma_start(out=bcr[:], in_=bc_d[:])
            nc.sync.dma_start(out=iota[:], in_=iota_d[:])
            ilo2 = cp.tile([128, CL2], i16)
            ihi2 = cp.tile([128, CH2], i16)
            wro2 = cp.tile([128, TT2], fp16)
            nc.scalar.dma_start(out=ilo2[:], in_=ilo2_d[:])
            nc.scalar.dma_start(out=ihi2[:], in_=ihi2_d[:])
            nc.scalar.dma_start(out=wro2[:], in_=wrow2_d[:])

            # replicated dis rows (dis precomputed on host), chunked to keep
            # the [1, *] scratch stripes small
            ones1 = cp.tile([1, 128], f32)
            nc.vector.memset(ones1[:], 1.0)
            disrep = cp.tile([128, NLOC], f32)
            c0 = 0
            while c0 < NLOC:
                cw = min(512, NLOC - c0)
                dch = wp.tile([1, 512], f32, tag="dch")
                nc.sync.dma_start(out=dch[:, :cw], in_=dis_d[0:1, c0:c0 + cw])
                ps = pp.tile([128, 512], f32, space="PSUM", tag="mm", bufs=3)
                nc.tensor.matmul(out=ps[:, :cw], lhsT=ones1[:],
                                 rhs=dch[0:1, :cw], start=True, stop=True)
                nc.vector.tensor_copy(out=disrep[:, c0:c0 + cw], in_=ps[:, :cw])
                c0 += cw

            # ---- one aggregation layer over all windows
            def layer(meta, tab_lo, tab_hi, ilo, ihi, wro, own_tab,
                      emit_window, post_window=None):
                T_lo, T_hi = meta["T_lo"], meta["T_hi"]
                n_lo, n_hi = meta["n_lo"], meta["n_hi"]
                CLO = [8 * sum(T_lo[:w]) for w in range(WPC)]
                CHI = [8 * sum(T_hi[:w]) for w in range(WPC)]
                CT = [sum(T_lo[:w]) + sum(T_hi[:w]) for w in range(WPC)]
                qn = [0]
                for w in range(WPC):
                    tl, th = T_lo[w], T_hi[w]
                    Tw = tl + th
                    clo, chi, ct = CLO[w], CHI[w], CT[w]
                    msg = mp.tile([128, maxT, 128], fp16, tag="msg", bufs=7)
                    # single_packet coalesces a gather's descriptor stream into
                    # one SDMA packet (much better 256B-descriptor throughput);
                    # packets cap at 64 descriptors = 7 tiles per dma_gather.
                    # num_idxs only covers the real (max-over-cores) index
                    # count — the tile-roundup tail lanes emit 4-byte dummy
                    # descriptors instead of fetching pad rows.
                    GMAX = 7
                    for half, tn, nn_, tab, itab, coff, toff in (
                            (0, tl, n_lo[w], tab_lo, ilo, clo, 0),
                            (1, th, n_hi[w], tab_hi, ihi, chi, tl)):
                        for t0 in range(0, tn, GMAX):
                            tc_ = min(GMAX, tn - t0)
                            nidx = min(nn_ - t0 * 128, tc_ * 128)
                            nc.gpsimd.dma_gather(
                                msg[:, toff + t0:toff + t0 + tc_, :], tab,
                                itab[:, coff + t0 * 8:coff + (t0 + tc_) * 8],
                                nidx, nidx, D, queue_num=qn[0] % 4)
                            qn[0] += 1
                    mself = mp.tile([128, 128], fp16, tag="mself", bufs=6)
                    meng = nc.sync if w % 2 else nc.scalar
                    meng.dma_start(out=mself[:],
                                   in_=own_tab[w * 128:(w + 1) * 128, :])
                    pa = pp.tile([128, 128], f32, space="PSUM", tag="agg", bufs=3)
                    nc.tensor.matmul(out=pa[:], lhsT=mself[:], rhs=ident[:],
                                     start=True, stop=(Tw == 0))
                    SG = 8
                    for g0 in range(0, Tw, SG):
                        gk = min(SG, Tw - g0)
                        S = sp_.tile([128, SG * 128], fp16, tag="S", bufs=7)
                        nc.vector.tensor_tensor(
                            out=S[:, :gk * 128].rearrange("p (t d) -> p t d", t=gk),
                            in0=wro[:, ct + g0:ct + g0 + gk].to_broadcast([128, gk, 128]),
                            in1=iota[:, :gk * 128].rearrange("p (t d) -> p t d", t=gk),
                            op=mybir.AluOpType.is_equal)
                        for t in range(g0, g0 + gk):
                            ts_ = t - g0
                            nc.tensor.matmul(
                                out=pa[:], lhsT=msg[:, t, :],
                                rhs=S[:, ts_ * 128:(ts_ + 1) * 128],
                                start=False, stop=(t == Tw - 1))
                    emit_window(w, pa)
                    if post_window is not None:
                        post_window(w)

            # zero the msg pool once: call-trimmed tail tiles are consumed by
            # the aggregation matmul (with zero S columns) before any gather
            # has written them, and SBUF must not hold NaN bit patterns there
            for b in range(7):
                mz = mp.tile([128, maxT, 128], fp16, tag="msg", bufs=7)
                nc.vector.memset(mz[:], 0.0)

            # layer 1 window epilogue, from pa = (A @ x-tilde) columns for
            # this window ([in_feat, dst]):
            #   z = dis*pa ; p1 = W1^T @ z  ([hid, dst]) ;
            #   h2 = relu(p1 + b1) ; y = dis*h2 ; htilde2 = y^T @ W2 -> ag_in
            def epi1(w, pa):
                dw = disrep[:, w * 128:(w + 1) * 128]
                z = wp.tile([128, 128], fp16, tag="z")
                nc.vector.tensor_mul(out=z[:], in0=pa[:], in1=dw)
                p1 = pp.tile([128, 128], f32, space="PSUM", tag="mm", bufs=3)
                nc.tensor.matmul(out=p1[:], lhsT=W1s[:], rhs=z[:],
                                 start=True, stop=True)
                h2 = wp.tile([128, 128], f32, tag="h2")
                nc.scalar.activation(h2[:], p1[:], mybir.ActivationFunctionType.Relu,
                                     bias=b1c[:, 0:1], scale=1.0)
                y = wp.tile([128, 128], fp16, tag="y")
                nc.vector.tensor_mul(out=y[:], in0=h2[:], in1=dw)
                p2 = pp.tile([128, 128], f32, space="PSUM", tag="mm", bufs=3)
                nc.tensor.matmul(out=p2[:], lhsT=y[:], rhs=W2s[:], start=True, stop=True)
                hb = wp.tile([128, 128], fp16, tag="hb")
                nc.vector.tensor_copy(out=hb[:], in_=p2[:])
                nc.sync.dma_start(out=ag_in[w * 128:(w + 1) * 128, :], in_=hb[:])

            # chunked AllGather into ag_all (chunk-major, core-major), each
            # chunk issued right after its last contributing window
            def emit_ag(k):
                a, b = spl[k] * 128, spl[k + 1] * 128
                base = 0
                for j in range(k):
                    base += n_cores * (spl[j + 1] - spl[j]) * 128
                nc.gpsimd.collective_compute(
                    "AllGather", mybir.AluOpType.bypass,
                    replica_groups=[list(range(n_cores))],
                    ins=[ag_in[a:b, :].opt()],
                    outs=[ag_all[base:base + n_cores * (b - a), :].opt()])

            ag_last = {}
            for k in range(len(spl) - 1):
                ag_last.setdefault(spl[k + 1] - 1, []).append(k)

            def post1(w):
                for k in ag_last.get(w, []):
                    emit_ag(k)

            layer(m1, xtab_lo_d[:], xtab_hi_d[:], ilo1, ihi1, wro1, xown_d,
                  epi1, post_window=post1)

            # layer 2 window epilogue: out3 = dis*agg + b2 ; out = out3^T@Wc + bc
            outacc = cp.tile([128, WPC, 2], f32)

            def epi2(w, pa):
                dw = disrep[:, w * 128:(w + 1) * 128]
                z = wp.tile([128, 128], f32, tag="z2")
                nc.vector.tensor_mul(out=z[:], in0=pa[:], in1=dw)
                o3 = wp.tile([128, 128], fp16, tag="o3")
                nc.scalar.activation(o3[:], z[:], mybir.ActivationFunctionType.Identity,
                                     bias=b2c[:, 0:1], scale=1.0)
                p3 = pp.tile([128, 2], f32, space="PSUM", tag="cls")
                nc.tensor.matmul(out=p3[:], lhsT=o3[:], rhs=Wcs[:], start=True, stop=True)
                nc.vector.tensor_add(out=outacc[:, w, :], in0=p3[:], in1=bcr[:])

            layer(m2, ag_all[0:LO, :], ag_all[LO:, :], ilo2, ihi2, wro2,
                  ag_in, epi2)
            nc.sync.dma_start(
                out=out_d[:WPC * 128, :].rearrange("(w p) c -> p w c", p=128),
                in_=outacc[:])

    nc.compile()
    return nc


# ------------------------------------------------------------------ driver

_CACHE = {}


def _get_program(struct):
    key = tuple(sorted((k, str(v)) for k, v in struct.items()))
    if key not in _CACHE:
        _CACHE[key] = build(struct)
    return _CACHE[key]


def kernel(x, edge_index, W1, b1, W2, b2, Wc, bc):
    x = np.asarray(x)
    N = x.shape[0]
    struct, per_core, xrows = prep(x, edge_index)
    nc = _get_program(struct)
    locN, NLOC = struct["locN"], struct["NLOC"]
    LO = struct["lo_rows"]

    common = dict(
        xtab_lo=xrows[:LO],
        xtab_hi=xrows[LO:],
        W1h=np.asarray(W1, np.float16),
        W2h=np.asarray(W2, np.float16),
        Wch=np.asarray(Wc, np.float16),
        b1c=np.asarray(b1, np.float32).reshape(D, 1),
        b2c=np.asarray(b2, np.float32).reshape(D, 1),
        bcrep=np.tile(np.asarray(bc, np.float32).reshape(1, 2), (D, 1)),
        iota=np.tile(np.arange(D, dtype=np.float16), (D, 8)),
        ident=np.eye(D, dtype=np.float16),
    )
    in_maps = []
    for c in range(N_CORES):
        m = dict(common)
        m["dis_row"] = per_core[c]["dis_row"]
        m["xown"] = per_core[c]["xown"]
        for kk in ("idx_lo1", "idx_hi1", "wrow1",
                   "idx_lo2", "idx_hi2", "wrow2"):
            m[kk] = per_core[c][kk]
        in_maps.append(m)

    trace = bool(int(os.environ.get("KERNEL_TRACE", "0")))
    res = run_bass_kernel_spmd(nc, in_maps, core_ids=list(range(N_CORES)),
                               trace=trace)
    globals()["_LAST_RES"] = res
    if trace and res.exec_time_ns is not None:
        print(f"HW exec time: {res.exec_time_ns} ns", flush=True)

    out = np.empty((N, 2), np.float32)
    for c in range(N_CORES):
        out[c * locN:(c + 1) * locN] = \
            res.results[c]["out"][per_core[c]["permrow"]]
    return out
